# revision 55
# baseline (speedup 1.0000x reference)
"""CopyGenerator kernel for 8x Trainium2 NeuronCores (Bass/Tile).

Computation (see reference):
    logits = hidden @ W.T + b            [BT, V]   (pad column masked to -inf)
    prob   = softmax(logits, axis=1)
    p_copy = sigmoid(hidden @ w_copy + b_copy)
    out    = concat([prob * (1 - p_copy),
                     einsum('bts,bsc', attn*p_copy, src_map)], axis=1)

Sharding: vocab dim of W/b/out_prob split 8 ways (tensor parallel).
All operand transposes are done on the host (free).  W^T stays resident
in SBUF (fp8, pre-scaled by 32), exp(logits) stays in SBUF (bf16).

v2 layout (per 128-token tile, vocab shard 6272 cols = 6x1024 + 128):
  PE   : 4 DoubleRow fp8 matmuls per 512-col bank, PSUM organized as
         4 rotating [128,1024] slots (8 banks total).
  DVE  : adds the (pre-scaled) bias row to each unit, PSUM -> SBUF f32
         staging chunks; frees the PSUM slot early so PE never waits.
  ACT  : exp over big staging chunks (2048/2176 wide) with accum_out
         row-sums; far fewer instructions than per-bank exp.
  pass2: after the normalizer AllReduce, out = exp * (1-p_copy)/S in 4
         chunks of 1568, split between DVE (tensor_scalar) and ACT
         (Relu activation with per-partition scale - same table set as
         exp, values are nonnegative so Relu is a scaled copy).
  p_copy matmul sweep + bias broadcast + copy branch run at startup in
  the shadow of the W DMA stream (W arrives in 1024-col chunks so the
  first matmuls start at ~4us).
"""

import sys

for _p in ("/opt/trn_rl_repo", "/root/.axon_site/_ro/trn_rl_repo"):
    if _p not in sys.path:
        sys.path.insert(0, _p)

import numpy as np

import concourse.bass as bass
import concourse.mybir as mybir
from concourse import bacc, tile
from concourse.bass_utils import run_bass_kernel_spmd

f32 = mybir.dt.float32
bf16 = mybir.dt.bfloat16
f8 = mybir.dt.float8e4
DR = mybir.MatmulPerfMode.DoubleRow
# SwInterleave: host pre-interleaves the stationary (A/B pairs per column,
# columns reversed) so LDWEIGHTS reads contiguously - avoids DoubleRow's
# +72% weight-load penalty (see trainium-docs tensor-engine guide).
SWI = True
DRS = mybir.MatmulPerfMode.DoubleRowSwInterleave if SWI else DR
P = 128
NK2 = 4                      # DoubleRow k-pairs (each covers 256 of D)
WSCALE = 32.0                # W/w_copy pre-scale into fp8 range
INV = 1.0 / WSCALE

B, T, S, C, V, D = 16, 128, 512, 512, 50000, 1024
BT = B * T
NCORES = 8
VSH = V // NCORES            # 6250 vocab columns per core
VSHP = 6272                  # padded to 49*128 (pad cols get b=-1e30 -> exp=0)
NK = D // P                  # 8 contraction k-tiles
NT = BT // P                 # 16 token tiles
NS = S // P                  # 4 copy-branch contraction k-tiles
BSH = B // NCORES            # 2 batches per core (copy branch)
# token tiles per normalizer exchange; adjacent sums stay <= exp bufs - so
# pass2(g-1) emitted after phase_a(g) never exhausts the exp pool.
GROUPS = [4, 4, 4, 3, 1]
EXP_BUFS = 7

# vocab units per tile: 6 x 1024 + 1 x 128 (PSUM slots are [P,1024])
UNITS = [(u * 1024, 1024) for u in range(6)] + [(6144, 128)]
# staging chunks (SBUF f32) -> one exp activation each
CHUNKS = [(0, 2048), (2048, 2048), (4096, 2176)]
# pass2 output chunks: (start, width, engine) engine 0=DVE 1=ACT
P2CH = [(0, 2112, 0), (2112, 2112, 1), (4224, 2048, 0)]


# Normalizer exchange via remote_dma_broadcast: an XOR-slot all-to-all of
# the per-core partial sums + local adds.  Two scheduler/runtime quirks:
#  - the Tile no-exec scheduler can't model remote semaphore arrivals, so
#    the arrival-wait target lives in a register loaded from an input
#    tensor (reads 0 in the scheduler -> wait passes; real target on HW);
#  - allocated semaphores are NOT cleared at kernel start, so residual
#    counts from earlier executions must be cleared explicitly before the
#    first exchange (peers' first sends are ~45us in, so a start-of-kernel
#    clear wins the race comfortably).
# Even with both fixes (and with the desc-gen preps correctly emitted AFTER
# the lsg reduces, and the arrival wait attached directly to the consuming
# add) the exchange never became correct: with a broken wait the adds read
# pre-arrival garbage; with a working wait the kernel hangs at the first
# group.  That pattern says the transport itself never delivers all 8
# slots - consistent with the 8 ranks spanning two chips (LNC2: 4 ranks
# per chip), where XOR rdests with delta-tpb >= 4 are invalid without
# cross-chip routing ids.  Fixing that needs routing-id plumbing that was
# out of budget, so the exchange stays on collective_compute.
# Also tried: per-tile interleave of pass2 drains (327.5us, within the
# config's 314-329us noise band, no clear win) and a 6-group split with a
# tiny final group (334us - the extra AllReduce hit the end-of-kernel DMA
# flood).  Groups [3,4,3,3,3] with pass2-after-next-group measured best.
USE_RDMA = False
DEBUG_RCV = False


def build_nc():
    nc = bacc.Bacc(
        "TRN2", target_bir_lowering=False, debug=False, num_devices=NCORES
    )
    if USE_RDMA:
        rsem = nc.alloc_semaphore("nrm_rsem")
        lsem = nc.alloc_semaphore("nrm_lsem")
    # [tt][din][k*128+t] = hidden[tt*128+t, k*128+din]
    hT_d = nc.declare_dram_parameter("hT", [NT, P, D], f8, isOutput=False)
    # [k][din][v] = W_shard[v, k*128+din]
    wT_d = nc.declare_dram_parameter("wT", [NK2, P, 2, VSHP], f8, isOutput=False)
    # bias row pre-broadcast to all 128 partitions on the host
    bbc_d = nc.declare_dram_parameter("b_bcast", [P, VSHP], bf16, isOutput=False)
    wc_d = nc.declare_dram_parameter("w_copyT", [P, NK2, 2, 16], f8,
                                   isOutput=False)
    bcn_d = nc.declare_dram_parameter("bc_neg", [P, 1], f32, isOutput=False)
    # [ks][s][t] = attn_shard[t, ks*128+s]
    at_d = nc.declare_dram_parameter("attnT", [NS, P, BSH * T], bf16, isOutput=False)
    # [i*NS+ks][s][c] = src_map[i, ks*128+s, c]
    src_d = nc.declare_dram_parameter("srcm", [BSH * NS, P, C], bf16, isOutput=False)
    hcb_d = nc.declare_dram_parameter("hidden_cb", [BSH, P, D], bf16, isOutput=False)
    wc16_d = nc.declare_dram_parameter("w_copyT16", [P, NK], bf16, isOutput=False)
    tgt_d = nc.declare_dram_parameter("nrm_tgt", [1, 16], mybir.dt.int32,
                                      isOutput=False)
    if DEBUG_RCV:
        dbg_d = nc.declare_dram_parameter("dbg_rcv", [len(GROUPS), P, 32],
                                          f32, isOutput=True)
    out_p = nc.declare_dram_parameter("out_prob", [BT, VSHP], bf16, isOutput=True)
    out_c = nc.declare_dram_parameter("copy_prob", [BSH * T, C], bf16, isOutput=True)

    Exp = mybir.ActivationFunctionType.Exp
    Relu = mybir.ActivationFunctionType.Relu
    add = mybir.AluOpType.add
    mult = mybir.AluOpType.mult

    with tile.TileContext(nc, num_cores=NCORES) as tc:
        from contextlib import ExitStack

        with ExitStack() as stack:
            constp = stack.enter_context(tc.tile_pool(name="const", bufs=1))
            wpool = stack.enter_context(tc.tile_pool(name="wres", bufs=1))
            htp = stack.enter_context(tc.tile_pool(name="hT", bufs=1))
            sumsp = stack.enter_context(tc.tile_pool(name="sums", bufs=3))
            smallp = stack.enter_context(tc.tile_pool(name="small", bufs=8))
            lsgp = stack.enter_context(tc.tile_pool(name="lsg", bufs=4))
            cbp = stack.enter_context(tc.tile_pool(name="cb", bufs=1))
            stgp = stack.enter_context(tc.tile_pool(name="stg", bufs=2))
            expp = stack.enter_context(tc.tile_pool(name="exp", bufs=EXP_BUFS))
            outsp = stack.enter_context(tc.tile_pool(name="outst", bufs=2))
            psmm = stack.enter_context(
                tc.tile_pool(name="psum_mm", bufs=4, space="PSUM"))
            dramp = stack.enter_context(
                tc.tile_pool(name="ccdram", bufs=2 * len(GROUPS), space="DRAM"))
            rcvp = stack.enter_context(
                tc.tile_pool(name="nrmrcv", bufs=len(GROUPS)))
            nrmp = stack.enter_context(tc.tile_pool(name="nrmadd", bufs=2))

            # ---- tiny constants (DMAs issued first; all tiny) ----
            wcT = constp.tile([P, NK2, 2, 16], f8)
            nc.sync.dma_start(wcT[:, :, :, :], wc_d.ap())
            wcT16 = constp.tile([P, NK], bf16)
            nc.sync.dma_start(wcT16[:, :], wc16_d.ap())
            bcNeg = constp.tile([P, 1], f32)
            nc.sync.dma_start(bcNeg[:, :], bcn_d.ap())
            tgt_t = constp.tile([1, 16], mybir.dt.int32)
            nc.sync.dma_start(tgt_t[:, :], tgt_d.ap())
            if USE_RDMA:
                wait_reg = nc.vector.alloc_register("nrm_wait")
                nc.vector.reg_mov(wait_reg, 0)
                # sems carry residue from previous NEFF executions
                nc.gpsimd.sem_clear(rsem)
                nc.gpsimd.sem_clear(lsem)

            pcall = constp.tile([P, NT], f32)
            S_all = constp.tile([P, NT], f32)

            # ---- first hidden tile + first W unit go out first so the
            # first matmul can start at ~4us; bias broadcast right behind.
            hT_t = [None] * NT
            w_t = [[None] * NK2 for _ in range(len(UNITS))]

            def dma_h(tt):
                t_ = htp.tile([P, NK2, 2, P], f8, name=f"hT{tt}")
                nc.sync.dma_start(t_[:, :, :, :], hT_d.ap()[tt])
                hT_t[tt] = t_

            def dma_w(u):
                c0, uw = UNITS[u]
                for k in range(NK2):
                    wt = wpool.tile([P, 2, uw], f8, name=f"w{k}u{u}")
                    nc.sync.dma_start(wt[:, :, :], wT_d.ap()[k, :, :, c0:c0 + uw])
                    w_t[u][k] = wt

            dma_h(0)
            dma_w(0)
            b_bc = constp.tile([P, VSHP], bf16)
            nc.sync.dma_start(b_bc[:, :], bbc_d.ap())
            dma_h(1)
            dma_h(2)
            for u in range(1, len(UNITS)):
                dma_w(u)
            for tt in range(3, NT):
                dma_h(tt)

            # ---- copy-branch input DMAs (arrive after W; computed later)
            attnT = cbp.tile([P, NS, BSH * T], bf16)
            for ks in range(NS):
                nc.sync.dma_start(attnT[:, ks, :], at_d.ap()[ks])
            hcb_t, src_t = [], []
            for i in range(BSH):
                hcb = cbp.tile([P, D], bf16, name=f"hcb{i}")
                nc.sync.dma_start(hcb[:, :], hcb_d.ap()[i])
                hcb_t.append(hcb)
                srcT = cbp.tile([P, NS, C], bf16, name=f"src{i}")
                for ks in range(NS):
                    nc.sync.dma_start(srcT[:, ks, :], src_d.ap()[i * NS + ks])
                src_t.append(srcT)

            # ---- p_copy sweep for a range of tiles (PE, startup shadow) --
            def pc_sweep(tiles):
                for tt in tiles:
                    pps = psmm.tile([P, 1024], f32, tag="mm")
                    for k in range(NK2):
                        nc.tensor.matmul(
                            pps[:, 0:1], hT_t[tt][:, k, :, :],
                            wcT[:, k, :, 0:1],
                            start=(k == 0), stop=(k == NK2 - 1),
                            perf_mode=DRS)
                    nc.scalar.activation(pcall[:, tt:tt + 1], pps[:, 0:1], Exp,
                                         bias=bcNeg[:, :], scale=-INV)

            # ---- copy branch (PE+DVE, cheap; placed mid-kernel) ----
            def copy_branch():
                for i in range(BSH):
                    pps = psmm.tile([P, 1024], f32, tag="mm")
                    for k in range(NK):
                        nc.tensor.matmul(
                            pps[:, 0:1], hcb_t[i][:, k * P:(k + 1) * P],
                            wcT16[:, k:k + 1],
                            start=(k == 0), stop=(k == NK - 1))
                    ycb = smallp.tile([P, 1], f32, tag="sc")
                    nc.scalar.activation(ycb[:, :], pps[:, 0:1], Exp,
                                         bias=bcNeg[:, :], scale=-1.0)
                    t1 = smallp.tile([P, 1], f32, tag="sc")
                    nc.vector.tensor_scalar(t1[:, :], ycb[:, :], 1.0, None, add)
                    pcb = smallp.tile([P, 1], f32, tag="sc")
                    nc.vector.reciprocal(pcb[:, :], t1[:, :])
                    cps = psmm.tile([P, 1024], f32, tag="mm")
                    for ks in range(NS):
                        nc.tensor.matmul(
                            cps[:, 0:C], attnT[:, ks, i * P:(i + 1) * P],
                            src_t[i][:, ks, :],
                            start=(ks == 0), stop=(ks == NS - 1))
                    cstg = cbp.tile([P, C], bf16, name=f"cst{i}")
                    nc.vector.tensor_scalar(cstg[:, :], cps[:, 0:C], pcb[:, :],
                                            None, mult)
                    nc.sync.dma_start(out_c.ap()[i * P:(i + 1) * P, :],
                                      cstg[:, :])

            # ---- phase A pieces -------------------------------------------
            # matmuls for one (tile, unit) -> PSUM slot, then DVE bias-add
            # into the tile's staging chunk.
            stg_of = {}     # (tt) -> list of (chunk_tile, c0, cw)

            def mm_unit(tt, u, exp_t, sums):
                c0, uw = UNITS[u]
                slot = psmm.tile([P, 1024], f32, tag="mm")
                for k in range(NK2):
                    for s0 in range(0, uw, 512):
                        w_ = min(512, uw - s0)
                        nc.tensor.matmul(
                            slot[:, s0:s0 + w_], hT_t[tt][:, k, :, :],
                            w_t[u][k][:, :, s0:s0 + w_],
                            start=(k == 0), stop=(k == NK2 - 1),
                            perf_mode=DRS)
                # which staging chunk does this unit land in?
                ci = 0 if u < 2 else (1 if u < 4 else 2)
                ch0, chw = CHUNKS[ci]
                if c0 == ch0:   # first unit of chunk: allocate
                    stg = stgp.tile([P, 2176], f32, tag="stg")
                    stg_of[tt] = stg_of.get(tt, {})
                    stg_of[tt][ci] = stg
                stg = stg_of[tt][ci]
                off = c0 - ch0
                nc.vector.tensor_tensor(
                    stg[:, off:off + uw], slot[:, :uw], b_bc[:, c0:c0 + uw],
                    add)
                # last unit of chunk: exp the whole chunk
                if c0 + uw == ch0 + chw:
                    nc.scalar.activation(
                        exp_t[:, ch0:ch0 + chw], stg[:, :chw], Exp, scale=INV,
                        accum_out=sums[:, ci:ci + 1])

            def phase_a(tt, lsg, j):
                exp_t = expp.tile([P, VSHP], bf16, tag="exp")
                sums = sumsp.tile([P, 3], f32, tag="sums")
                for u in range(len(UNITS)):
                    mm_unit(tt, u, exp_t, sums)
                nc.vector.tensor_reduce(lsg[:, j:j + 1], sums[:, :3],
                                        mybir.AxisListType.X, add)
                return exp_t

            # ---- pass 2: scale by (1-p_copy)/S and store ------------------
            def pass2(tt, exp_t):
                y = pcall[:, tt:tt + 1]
                # t2 = (1 + y) * S  computed as y*S + S in one tensor_scalar
                t2 = smallp.tile([P, 1], f32, tag="sc")
                nc.vector.tensor_scalar(t2[:, :], y, S_all[:, tt:tt + 1],
                                        S_all[:, tt:tt + 1], mult, add)
                t3 = smallp.tile([P, 1], f32, tag="sc")
                nc.vector.reciprocal(t3[:, :], t2[:, :])
                rs = smallp.tile([P, 1], f32, tag="sc")
                nc.vector.tensor_scalar(rs[:, :], t3[:, :], y, None, mult)
                for (c0, cw, eng) in P2CH:
                    ost = outsp.tile([P, 2112], bf16, tag="outst")
                    if eng == 0:
                        nc.vector.tensor_scalar(ost[:, :cw],
                                                exp_t[:, c0:c0 + cw],
                                                rs[:, :], None, mult)
                    else:
                        nc.scalar.activation(ost[:, :cw], exp_t[:, c0:c0 + cw],
                                             Relu, scale=rs[:, :])
                    nc.sync.dma_start(
                        out_p.ap()[tt * P:(tt + 1) * P, c0:c0 + cw],
                        ost[:, :cw])

            # ---- main schedule -------------------------------------------
            # pass2(g-1) is emitted only after all of phase_a(g), like the
            # baseline: this keeps every cross-engine wait pointing at
            # instructions EARLIER in each engine's stream (deadlock-free)
            # and gives each AllReduce a full group of compute to hide in.
            groups = []
            t0 = 0
            for gsz in GROUPS:
                groups.append(list(range(t0, t0 + gsz)))
                t0 += gsz
            assert t0 == NT

            exp_tiles = {}
            recv_tiles = []
            pending = []

            def drain_one():
                if pending:
                    tt = pending.pop(0)
                    pass2(tt, exp_tiles.pop(tt))

            for g, grp in enumerate(groups):
                G = len(grp)
                lsg = lsgp.tile([P, 4], f32, tag="lsg")
                if USE_RDMA:
                    # emit desc-gen early: Q7 descriptor generation overlaps
                    # this group's matmuls; the lsg read is deferred to the
                    # trigger below (Tile moves the data dep there).
                    recv = rcvp.tile([P, NCORES, 4], f32, tag="rcv")
                    recv_tiles.append(recv)
                    for k in range(NCORES):
                        rd = [None] * NCORES
                        rd[k] = (0, k)
                        nc.gpsimd.remote_dma_broadcast(
                            recv[:, k, 0:G], lsg[:, 0:G], rsem, lsem,
                            rdests=rd)
                for j, tt in enumerate(grp):
                    exp_tiles[tt] = phase_a(tt, lsg, j)
                    # per-tile pass2 drains keep exp in-flight <= 7 bufs
                    # with adjacent groups of 4; g==1 starts a tile later
                    # so AR0 (+11us warmup) has landed.
                    if (g >= 2 and j >= 1) or (g == 1 and j >= 2):
                        drain_one()
                if USE_RDMA:
                    nc.gpsimd.trigger_dma(count=None)
                else:
                    cc_in = dramp.tile([P, G], f32, tag="cc_in")
                    cc_out = dramp.tile([P, G], f32, tag="cc_out")
                    nc.sync.dma_start(cc_in[:, :], lsg[:, 0:G])
                    nc.gpsimd.collective_compute(
                        "AllReduce", add,
                        replica_groups=[list(range(NCORES))],
                        ins=[cc_in.opt()], outs=[cc_out.opt()],
                    )
                    nc.sync.dma_start(
                        S_all[:, grp[0]:grp[0] + G], cc_out[:, :])
                if g == 0:
                    # p_copy sweep fills the first exchange's latency shadow
                    pc_sweep(range(NT))
                if g == 1:
                    copy_branch()
                while pending:
                    drain_one()
                pending.extend(grp)
                if USE_RDMA:
                    # arrivals landed while pass2(g-1) ran; sum the 8
                    # per-core partials into S_all for this group.  The wait
                    # target comes from a data load (see USE_RDMA comment).
                    nc.vector.reg_load(wait_reg, tgt_t[0:1, g:g + 1])
                    nc.vector.wait_ge(rsem, wait_reg)
                    cur = recv[:, 0, 0:G]
                    for k in range(1, NCORES):
                        if k == NCORES - 1:
                            nxt = S_all[:, grp[0]:grp[0] + G]
                        else:
                            nt_ = nrmp.tile([P, 4], f32, tag="nrm")
                            nxt = nt_[:, 0:G]
                        nc.vector.tensor_tensor(nxt, cur, recv[:, k, 0:G],
                                                add)
                        cur = nxt
            while pending:
                drain_one()
            if USE_RDMA and DEBUG_RCV:
                for g, recv in enumerate(recv_tiles):
                    nc.sync.dma_start(dbg_d.ap()[g], recv[:, :, :])

    nc.finalize()
    return nc


_CACHE = {}


def _get_nc():
    if "nc" not in _CACHE:
        _CACHE["nc"] = build_nc()
    return _CACHE["nc"]


def make_in_maps(hidden, attn, src_map, W, b, w_copy, b_copy, pad_idx):
    import ml_dtypes

    bF = ml_dtypes.bfloat16
    hidden = np.asarray(hidden, np.float32)
    attn = np.asarray(attn, np.float32)
    src_map = np.asarray(src_map, np.float32)
    W = np.asarray(W, np.float32)
    b = np.asarray(b, np.float32)
    w_copy = np.asarray(w_copy, np.float32)
    b_copy = np.asarray(b_copy, np.float32)
    pad = int(np.asarray(pad_idx))

    f8np = ml_dtypes.float8_e4m3fn
    # hidden^T DoubleRow tiles: [tt, din, q, i, t]
    H5 = hidden.reshape(NT, P, NK2, 2, P).transpose(0, 4, 2, 3, 1)
    if SWI:
        # SwInterleave stationary: flat[2j+i] = orig[i, 127-j]
        H5 = H5[..., ::-1].transpose(0, 1, 2, 4, 3)
    H3 = np.ascontiguousarray(H5.reshape(NT, P, D).astype(f8np))
    wc4 = np.zeros((P, NK2, 2, 16), np.float32)
    wc4[:, :, :, 0] = (w_copy * WSCALE).reshape(NK2, 2, P).transpose(2, 0, 1)
    wcT = np.ascontiguousarray(wc4.astype(f8np))
    wcT16 = np.ascontiguousarray(w_copy.reshape(NK, P).T.astype(bF))
    H16 = hidden.reshape(NT, P, NK, P).transpose(0, 3, 2, 1)
    H16 = np.ascontiguousarray(H16.reshape(NT, P, D).astype(bF))
    bcn = np.ascontiguousarray(
        np.full((P, 1), -float(b_copy[0]), np.float32))

    in_maps = []
    for c in range(NCORES):
        lo, hi = c * VSH, (c + 1) * VSH
        Wp = np.zeros((VSHP, D), np.float32)
        Wp[:VSH] = W[lo:hi] * WSCALE
        wT = np.ascontiguousarray(
            Wp.reshape(VSHP, NK2, 2, P).transpose(1, 3, 2, 0).astype(f8np))
        bsl = np.full((VSHP,), -1e30, np.float32)
        bsl[:VSH] = b[lo:hi] * WSCALE
        if lo <= pad < hi:
            bsl[pad - lo] = -1e30
        b_bcast = np.ascontiguousarray(
            np.broadcast_to(bsl.astype(bF).reshape(1, VSHP), (P, VSHP)))
        a_sl = attn[c * BSH * T:(c + 1) * BSH * T]
        attnT = np.ascontiguousarray(
            a_sl.reshape(BSH * T, NS, P).transpose(1, 2, 0).astype(bF))
        s_sl = src_map[c * BSH:(c + 1) * BSH]
        srcm = np.ascontiguousarray(
            s_sl.reshape(BSH * NS, P, C).astype(bF))
        tgts = np.zeros((1, 16), np.int32)
        for g in range(len(GROUPS)):
            tgts[0, g] = 2 * NCORES * (g + 1)
        in_maps.append({
            "hT": H3,
            "wT": wT,
            "b_bcast": b_bcast,
            "w_copyT": wcT,
            "bc_neg": bcn,
            "attnT": attnT,
            "srcm": srcm,
            "hidden_cb": np.ascontiguousarray(H16[c * BSH:(c + 1) * BSH]),
            "w_copyT16": wcT16,
            "nrm_tgt": tgts,
        })
    return in_maps


def assemble(results):
    out_prob = np.concatenate(
        [np.asarray(r["out_prob"], np.float32)[:, :VSH] for r in results],
        axis=1)
    copy_prob = np.concatenate(
        [np.asarray(r["copy_prob"]).astype(np.float32) for r in results],
        axis=0)
    return np.concatenate([out_prob, copy_prob], axis=1)


FULL_CFG = dict(B=B, T=T, S=S, C=C, V=V, D=D)


def run(cfg, inputs, trace=False):
    """test.py interface: run(K.FULL_CFG, np_inputs, trace=True)."""
    nc = _get_nc()
    in_maps = make_in_maps(**inputs)
    res = run_bass_kernel_spmd(nc, in_maps, list(range(NCORES)), trace=trace)
    return assemble(res.results), res


def kernel(**inputs) -> np.ndarray:
    out, _ = run(FULL_CFG, inputs, trace=False)
    return out


# revision 57
# speedup vs baseline: 1.0841x; 1.0841x over previous
"""CopyGenerator kernel for 8x Trainium2 NeuronCores (Bass/Tile).

Computation (see reference):
    logits = hidden @ W.T + b            [BT, V]   (pad column masked to -inf)
    prob   = softmax(logits, axis=1)
    p_copy = sigmoid(hidden @ w_copy + b_copy)
    out    = concat([prob * (1 - p_copy),
                     einsum('bts,bsc', attn*p_copy, src_map)], axis=1)

Sharding: vocab dim of W/b/out_prob split 8 ways (tensor parallel).
All operand transposes are done on the host (free).  W^T stays resident
in SBUF (fp8, pre-scaled by 32), exp(logits) stays in SBUF (bf16).

v2 layout (per 128-token tile, vocab shard 6272 cols = 6x1024 + 128):
  PE   : 4 DoubleRow fp8 matmuls per 512-col bank, PSUM organized as
         4 rotating [128,1024] slots (8 banks total).
  DVE  : adds the (pre-scaled) bias row to each unit, PSUM -> SBUF f32
         staging chunks; frees the PSUM slot early so PE never waits.
  ACT  : exp over big staging chunks (2048/2176 wide) with accum_out
         row-sums; far fewer instructions than per-bank exp.
  pass2: after the normalizer AllReduce, out = exp * (1-p_copy)/S in 4
         chunks of 1568, split between DVE (tensor_scalar) and ACT
         (Relu activation with per-partition scale - same table set as
         exp, values are nonnegative so Relu is a scaled copy).
  p_copy matmul sweep + bias broadcast + copy branch run at startup in
  the shadow of the W DMA stream (W arrives in 1024-col chunks so the
  first matmuls start at ~4us).
"""

import sys

for _p in ("/opt/trn_rl_repo", "/root/.axon_site/_ro/trn_rl_repo"):
    if _p not in sys.path:
        sys.path.insert(0, _p)

import numpy as np

import concourse.bass as bass
import concourse.mybir as mybir
from concourse import bacc, tile
from concourse.bass_utils import run_bass_kernel_spmd

f32 = mybir.dt.float32
bf16 = mybir.dt.bfloat16
f8 = mybir.dt.float8e4
DR = mybir.MatmulPerfMode.DoubleRow
# SwInterleave: host pre-interleaves the stationary (A/B pairs per column,
# columns reversed) so LDWEIGHTS reads contiguously - avoids DoubleRow's
# +72% weight-load penalty (see trainium-docs tensor-engine guide).
SWI = True
DRS = mybir.MatmulPerfMode.DoubleRowSwInterleave if SWI else DR
P = 128
NK2 = 4                      # DoubleRow k-pairs (each covers 256 of D)
WSCALE = 32.0                # W/w_copy pre-scale into fp8 range
INV = 1.0 / WSCALE

B, T, S, C, V, D = 16, 128, 512, 512, 50000, 1024
BT = B * T
NCORES = 8
VSH = V // NCORES            # 6250 vocab columns per core
VSHP = 6272                  # padded to 49*128 (pad cols get b=-1e30 -> exp=0)
NK = D // P                  # 8 contraction k-tiles
NT = BT // P                 # 16 token tiles
NS = S // P                  # 4 copy-branch contraction k-tiles
BSH = B // NCORES            # 2 batches per core (copy branch)
# token tiles per normalizer exchange; adjacent sums stay <= exp bufs - so
# pass2(g-1) emitted after phase_a(g) never exhausts the exp pool.
GROUPS = [3, 4, 3, 3, 3]
EXP_BUFS = 7

# vocab units per tile: 6 x 1024 + 1 x 128 (PSUM slots are [P,1024])
UNITS = [(u * 1024, 1024) for u in range(6)] + [(6144, 128)]
# staging chunks (SBUF f32) -> one exp activation each
CHUNKS = [(0, 2048), (2048, 2048), (4096, 2176)]
# pass2 output chunks: (start, width, engine) engine 0=DVE 1=ACT
P2CH = [(0, 2112, 0), (2112, 2112, 1), (4224, 2048, 0)]


# Normalizer exchange via remote_dma_broadcast: an XOR-slot all-to-all of
# the per-core partial sums + local adds.  Two scheduler/runtime quirks:
#  - the Tile no-exec scheduler can't model remote semaphore arrivals, so
#    the arrival-wait target lives in a register loaded from an input
#    tensor (reads 0 in the scheduler -> wait passes; real target on HW);
#  - allocated semaphores are NOT cleared at kernel start, so residual
#    counts from earlier executions must be cleared explicitly before the
#    first exchange (peers' first sends are ~45us in, so a start-of-kernel
#    clear wins the race comfortably).
# Even with both fixes (and with the desc-gen preps correctly emitted AFTER
# the lsg reduces, and the arrival wait attached directly to the consuming
# add) the exchange never became correct: with a broken wait the adds read
# pre-arrival garbage; with a working wait the kernel hangs at the first
# group.  That pattern says the transport itself never delivers all 8
# slots - consistent with the 8 ranks spanning two chips (LNC2: 4 ranks
# per chip), where XOR rdests with delta-tpb >= 4 are invalid without
# cross-chip routing ids.  Fixing that needs routing-id plumbing that was
# out of budget, so the exchange stays on collective_compute.
# Also tried: per-tile interleave of pass2 drains (327.5us, within the
# config's 314-329us noise band, no clear win) and a 6-group split with a
# tiny final group (334us - the extra AllReduce hit the end-of-kernel DMA
# flood).  Groups [3,4,3,3,3] with pass2-after-next-group measured best.
USE_RDMA = False
DEBUG_RCV = False


def build_nc():
    nc = bacc.Bacc(
        "TRN2", target_bir_lowering=False, debug=False, num_devices=NCORES
    )
    if USE_RDMA:
        rsem = nc.alloc_semaphore("nrm_rsem")
        lsem = nc.alloc_semaphore("nrm_lsem")
    # [tt][din][k*128+t] = hidden[tt*128+t, k*128+din]
    hT_d = nc.declare_dram_parameter("hT", [NT, P, D], f8, isOutput=False)
    # [k][din][v] = W_shard[v, k*128+din]
    wT_d = nc.declare_dram_parameter("wT", [NK2, P, 2, VSHP], f8, isOutput=False)
    # bias row pre-broadcast to all 128 partitions on the host
    bbc_d = nc.declare_dram_parameter("b_bcast", [P, VSHP], bf16, isOutput=False)
    wc_d = nc.declare_dram_parameter("w_copyT", [P, NK2, 2, 16], f8,
                                   isOutput=False)
    bcn_d = nc.declare_dram_parameter("bc_neg", [P, 1], f32, isOutput=False)
    # [ks][s][t] = attn_shard[t, ks*128+s]
    at_d = nc.declare_dram_parameter("attnT", [NS, P, BSH * T], bf16, isOutput=False)
    # [i*NS+ks][s][c] = src_map[i, ks*128+s, c]
    src_d = nc.declare_dram_parameter("srcm", [BSH * NS, P, C], bf16, isOutput=False)
    hcb_d = nc.declare_dram_parameter("hidden_cb", [BSH, P, D], bf16, isOutput=False)
    wc16_d = nc.declare_dram_parameter("w_copyT16", [P, NK], bf16, isOutput=False)
    tgt_d = nc.declare_dram_parameter("nrm_tgt", [1, 16], mybir.dt.int32,
                                      isOutput=False)
    if DEBUG_RCV:
        dbg_d = nc.declare_dram_parameter("dbg_rcv", [len(GROUPS), P, 32],
                                          f32, isOutput=True)
    out_p = nc.declare_dram_parameter("out_prob", [BT, VSHP], bf16, isOutput=True)
    out_c = nc.declare_dram_parameter("copy_prob", [BSH * T, C], bf16, isOutput=True)

    Exp = mybir.ActivationFunctionType.Exp
    Relu = mybir.ActivationFunctionType.Relu
    add = mybir.AluOpType.add
    mult = mybir.AluOpType.mult

    with tile.TileContext(nc, num_cores=NCORES) as tc:
        from contextlib import ExitStack

        with ExitStack() as stack:
            constp = stack.enter_context(tc.tile_pool(name="const", bufs=1))
            wpool = stack.enter_context(tc.tile_pool(name="wres", bufs=1))
            htp = stack.enter_context(tc.tile_pool(name="hT", bufs=1))
            sumsp = stack.enter_context(tc.tile_pool(name="sums", bufs=3))
            smallp = stack.enter_context(tc.tile_pool(name="small", bufs=8))
            lsgp = stack.enter_context(tc.tile_pool(name="lsg", bufs=4))
            cbp = stack.enter_context(tc.tile_pool(name="cb", bufs=1))
            stgp = stack.enter_context(tc.tile_pool(name="stg", bufs=2))
            expp = stack.enter_context(tc.tile_pool(name="exp", bufs=EXP_BUFS))
            outsp = stack.enter_context(tc.tile_pool(name="outst", bufs=2))
            psmm = stack.enter_context(
                tc.tile_pool(name="psum_mm", bufs=4, space="PSUM"))
            dramp = stack.enter_context(
                tc.tile_pool(name="ccdram", bufs=2 * len(GROUPS), space="DRAM"))
            rcvp = stack.enter_context(
                tc.tile_pool(name="nrmrcv", bufs=len(GROUPS)))
            nrmp = stack.enter_context(tc.tile_pool(name="nrmadd", bufs=2))

            # ---- tiny constants (DMAs issued first; all tiny) ----
            wcT = constp.tile([P, NK2, 2, 16], f8)
            nc.sync.dma_start(wcT[:, :, :, :], wc_d.ap())
            wcT16 = constp.tile([P, NK], bf16)
            nc.sync.dma_start(wcT16[:, :], wc16_d.ap())
            bcNeg = constp.tile([P, 1], f32)
            nc.sync.dma_start(bcNeg[:, :], bcn_d.ap())
            tgt_t = constp.tile([1, 16], mybir.dt.int32)
            nc.sync.dma_start(tgt_t[:, :], tgt_d.ap())
            if USE_RDMA:
                wait_reg = nc.vector.alloc_register("nrm_wait")
                nc.vector.reg_mov(wait_reg, 0)
                # sems carry residue from previous NEFF executions
                nc.gpsimd.sem_clear(rsem)
                nc.gpsimd.sem_clear(lsem)

            pcall = constp.tile([P, NT], f32)
            S_all = constp.tile([P, NT], f32)

            # ---- first hidden tile + first W unit go out first so the
            # first matmul can start at ~4us; bias broadcast right behind.
            hT_t = [None] * NT
            w_t = [[None] * NK2 for _ in range(len(UNITS))]

            def dma_h(tt):
                t_ = htp.tile([P, NK2, 2, P], f8, name=f"hT{tt}")
                nc.sync.dma_start(t_[:, :, :, :], hT_d.ap()[tt])
                hT_t[tt] = t_

            def dma_w(u):
                c0, uw = UNITS[u]
                for k in range(NK2):
                    wt = wpool.tile([P, 2, uw], f8, name=f"w{k}u{u}")
                    nc.sync.dma_start(wt[:, :, :], wT_d.ap()[k, :, :, c0:c0 + uw])
                    w_t[u][k] = wt

            dma_h(0)
            dma_w(0)
            b_bc = constp.tile([P, VSHP], bf16)
            nc.sync.dma_start(b_bc[:, :], bbc_d.ap())
            dma_h(1)
            dma_h(2)
            for u in range(1, len(UNITS)):
                dma_w(u)
            for tt in range(3, NT):
                dma_h(tt)

            # ---- copy-branch input DMAs (arrive after W; computed later)
            attnT = cbp.tile([P, NS, BSH * T], bf16)
            for ks in range(NS):
                nc.sync.dma_start(attnT[:, ks, :], at_d.ap()[ks])
            hcb_t, src_t = [], []
            for i in range(BSH):
                hcb = cbp.tile([P, D], bf16, name=f"hcb{i}")
                nc.sync.dma_start(hcb[:, :], hcb_d.ap()[i])
                hcb_t.append(hcb)
                srcT = cbp.tile([P, NS, C], bf16, name=f"src{i}")
                for ks in range(NS):
                    nc.sync.dma_start(srcT[:, ks, :], src_d.ap()[i * NS + ks])
                src_t.append(srcT)

            # ---- p_copy sweep for a range of tiles (PE, startup shadow) --
            def pc_sweep(tiles):
                for tt in tiles:
                    pps = psmm.tile([P, 1024], f32, tag="mm")
                    for k in range(NK2):
                        nc.tensor.matmul(
                            pps[:, 0:1], hT_t[tt][:, k, :, :],
                            wcT[:, k, :, 0:1],
                            start=(k == 0), stop=(k == NK2 - 1),
                            perf_mode=DRS)
                    nc.scalar.activation(pcall[:, tt:tt + 1], pps[:, 0:1], Exp,
                                         bias=bcNeg[:, :], scale=-INV)

            # ---- copy branch (PE+DVE, cheap; placed mid-kernel) ----
            def copy_branch():
                for i in range(BSH):
                    pps = psmm.tile([P, 1024], f32, tag="mm")
                    for k in range(NK):
                        nc.tensor.matmul(
                            pps[:, 0:1], hcb_t[i][:, k * P:(k + 1) * P],
                            wcT16[:, k:k + 1],
                            start=(k == 0), stop=(k == NK - 1))
                    ycb = smallp.tile([P, 1], f32, tag="sc")
                    nc.scalar.activation(ycb[:, :], pps[:, 0:1], Exp,
                                         bias=bcNeg[:, :], scale=-1.0)
                    t1 = smallp.tile([P, 1], f32, tag="sc")
                    nc.vector.tensor_scalar(t1[:, :], ycb[:, :], 1.0, None, add)
                    pcb = smallp.tile([P, 1], f32, tag="sc")
                    nc.vector.reciprocal(pcb[:, :], t1[:, :])
                    cps = psmm.tile([P, 1024], f32, tag="mm")
                    for ks in range(NS):
                        nc.tensor.matmul(
                            cps[:, 0:C], attnT[:, ks, i * P:(i + 1) * P],
                            src_t[i][:, ks, :],
                            start=(ks == 0), stop=(ks == NS - 1))
                    cstg = cbp.tile([P, C], bf16, name=f"cst{i}")
                    nc.vector.tensor_scalar(cstg[:, :], cps[:, 0:C], pcb[:, :],
                                            None, mult)
                    nc.sync.dma_start(out_c.ap()[i * P:(i + 1) * P, :],
                                      cstg[:, :])

            # ---- phase A pieces -------------------------------------------
            # matmuls for one (tile, unit) -> PSUM slot, then DVE bias-add
            # into the tile's staging chunk.
            stg_of = {}     # (tt) -> list of (chunk_tile, c0, cw)

            def mm_unit(tt, u, exp_t, sums):
                c0, uw = UNITS[u]
                slot = psmm.tile([P, 1024], f32, tag="mm")
                for k in range(NK2):
                    for s0 in range(0, uw, 512):
                        w_ = min(512, uw - s0)
                        nc.tensor.matmul(
                            slot[:, s0:s0 + w_], hT_t[tt][:, k, :, :],
                            w_t[u][k][:, :, s0:s0 + w_],
                            start=(k == 0), stop=(k == NK2 - 1),
                            perf_mode=DRS)
                # which staging chunk does this unit land in?
                ci = 0 if u < 2 else (1 if u < 4 else 2)
                ch0, chw = CHUNKS[ci]
                if c0 == ch0:   # first unit of chunk: allocate
                    stg = stgp.tile([P, 2176], f32, tag="stg")
                    stg_of[tt] = stg_of.get(tt, {})
                    stg_of[tt][ci] = stg
                stg = stg_of[tt][ci]
                off = c0 - ch0
                nc.vector.tensor_tensor(
                    stg[:, off:off + uw], slot[:, :uw], b_bc[:, c0:c0 + uw],
                    add)
                # last unit of chunk: exp the whole chunk
                if c0 + uw == ch0 + chw:
                    nc.scalar.activation(
                        exp_t[:, ch0:ch0 + chw], stg[:, :chw], Exp, scale=INV,
                        accum_out=sums[:, ci:ci + 1])

            def phase_a(tt, lsg, j):
                exp_t = expp.tile([P, VSHP], bf16, tag="exp")
                sums = sumsp.tile([P, 3], f32, tag="sums")
                for u in range(len(UNITS)):
                    mm_unit(tt, u, exp_t, sums)
                nc.vector.tensor_reduce(lsg[:, j:j + 1], sums[:, :3],
                                        mybir.AxisListType.X, add)
                return exp_t

            # ---- pass 2: scale by (1-p_copy)/S and store ------------------
            def pass2(tt, exp_t):
                y = pcall[:, tt:tt + 1]
                # t2 = (1 + y) * S  computed as y*S + S in one tensor_scalar
                t2 = smallp.tile([P, 1], f32, tag="sc")
                nc.vector.tensor_scalar(t2[:, :], y, S_all[:, tt:tt + 1],
                                        S_all[:, tt:tt + 1], mult, add)
                t3 = smallp.tile([P, 1], f32, tag="sc")
                nc.vector.reciprocal(t3[:, :], t2[:, :])
                rs = smallp.tile([P, 1], f32, tag="sc")
                nc.vector.tensor_scalar(rs[:, :], t3[:, :], y, None, mult)
                for (c0, cw, eng) in P2CH:
                    ost = outsp.tile([P, 2112], bf16, tag="outst")
                    if eng == 0:
                        nc.vector.tensor_scalar(ost[:, :cw],
                                                exp_t[:, c0:c0 + cw],
                                                rs[:, :], None, mult)
                    else:
                        nc.scalar.activation(ost[:, :cw], exp_t[:, c0:c0 + cw],
                                             Relu, scale=rs[:, :])
                    nc.sync.dma_start(
                        out_p.ap()[tt * P:(tt + 1) * P, c0:c0 + cw],
                        ost[:, :cw])

            # ---- main schedule -------------------------------------------
            # pass2(g-1) is emitted only after all of phase_a(g), like the
            # baseline: this keeps every cross-engine wait pointing at
            # instructions EARLIER in each engine's stream (deadlock-free)
            # and gives each AllReduce a full group of compute to hide in.
            groups = []
            t0 = 0
            for gsz in GROUPS:
                groups.append(list(range(t0, t0 + gsz)))
                t0 += gsz
            assert t0 == NT

            exp_tiles = {}
            recv_tiles = []
            for g, grp in enumerate(groups):
                G = len(grp)
                lsg = lsgp.tile([P, 4], f32, tag="lsg")
                if USE_RDMA:
                    # emit desc-gen early: Q7 descriptor generation overlaps
                    # this group's matmuls; the lsg read is deferred to the
                    # trigger below (Tile moves the data dep there).
                    recv = rcvp.tile([P, NCORES, 4], f32, tag="rcv")
                    recv_tiles.append(recv)
                    for k in range(NCORES):
                        rd = [None] * NCORES
                        rd[k] = (0, k)
                        nc.gpsimd.remote_dma_broadcast(
                            recv[:, k, 0:G], lsg[:, 0:G], rsem, lsem,
                            rdests=rd)
                for j, tt in enumerate(grp):
                    exp_tiles[tt] = phase_a(tt, lsg, j)
                if USE_RDMA:
                    nc.gpsimd.trigger_dma(count=None)
                else:
                    cc_in = dramp.tile([P, G], f32, tag="cc_in")
                    cc_out = dramp.tile([P, G], f32, tag="cc_out")
                    nc.sync.dma_start(cc_in[:, :], lsg[:, 0:G])
                    nc.gpsimd.collective_compute(
                        "AllReduce", add,
                        replica_groups=[list(range(NCORES))],
                        ins=[cc_in.opt()], outs=[cc_out.opt()],
                    )
                    nc.sync.dma_start(
                        S_all[:, grp[0]:grp[0] + G], cc_out[:, :])
                if g == 0:
                    # p_copy sweep fills the first exchange's latency shadow
                    pc_sweep(range(NT))
                if g == len(groups) - 1:
                    # the PE is saturated mid-kernel but idle during the
                    # final AllReduce's flight - the copy branch is free here
                    copy_branch()
                if g >= 1:
                    for tt in groups[g - 1]:
                        pass2(tt, exp_tiles.pop(tt))
                if USE_RDMA:
                    # arrivals landed while pass2(g-1) ran; sum the 8
                    # per-core partials into S_all for this group.  The wait
                    # target comes from a data load (see USE_RDMA comment).
                    nc.vector.reg_load(wait_reg, tgt_t[0:1, g:g + 1])
                    nc.vector.wait_ge(rsem, wait_reg)
                    cur = recv[:, 0, 0:G]
                    for k in range(1, NCORES):
                        if k == NCORES - 1:
                            nxt = S_all[:, grp[0]:grp[0] + G]
                        else:
                            nt_ = nrmp.tile([P, 4], f32, tag="nrm")
                            nxt = nt_[:, 0:G]
                        nc.vector.tensor_tensor(nxt, cur, recv[:, k, 0:G],
                                                add)
                        cur = nxt
            for tt in groups[-1]:
                pass2(tt, exp_tiles.pop(tt))
            if USE_RDMA and DEBUG_RCV:
                for g, recv in enumerate(recv_tiles):
                    nc.sync.dma_start(dbg_d.ap()[g], recv[:, :, :])

    nc.finalize()
    return nc


_CACHE = {}


def _get_nc():
    if "nc" not in _CACHE:
        _CACHE["nc"] = build_nc()
    return _CACHE["nc"]


def make_in_maps(hidden, attn, src_map, W, b, w_copy, b_copy, pad_idx):
    import ml_dtypes

    bF = ml_dtypes.bfloat16
    hidden = np.asarray(hidden, np.float32)
    attn = np.asarray(attn, np.float32)
    src_map = np.asarray(src_map, np.float32)
    W = np.asarray(W, np.float32)
    b = np.asarray(b, np.float32)
    w_copy = np.asarray(w_copy, np.float32)
    b_copy = np.asarray(b_copy, np.float32)
    pad = int(np.asarray(pad_idx))

    f8np = ml_dtypes.float8_e4m3fn
    # hidden^T DoubleRow tiles: [tt, din, q, i, t]
    H5 = hidden.reshape(NT, P, NK2, 2, P).transpose(0, 4, 2, 3, 1)
    if SWI:
        # SwInterleave stationary: flat[2j+i] = orig[i, 127-j]
        H5 = H5[..., ::-1].transpose(0, 1, 2, 4, 3)
    H3 = np.ascontiguousarray(H5.reshape(NT, P, D).astype(f8np))
    wc4 = np.zeros((P, NK2, 2, 16), np.float32)
    wc4[:, :, :, 0] = (w_copy * WSCALE).reshape(NK2, 2, P).transpose(2, 0, 1)
    wcT = np.ascontiguousarray(wc4.astype(f8np))
    wcT16 = np.ascontiguousarray(w_copy.reshape(NK, P).T.astype(bF))
    H16 = hidden.reshape(NT, P, NK, P).transpose(0, 3, 2, 1)
    H16 = np.ascontiguousarray(H16.reshape(NT, P, D).astype(bF))
    bcn = np.ascontiguousarray(
        np.full((P, 1), -float(b_copy[0]), np.float32))

    in_maps = []
    for c in range(NCORES):
        lo, hi = c * VSH, (c + 1) * VSH
        Wp = np.zeros((VSHP, D), np.float32)
        Wp[:VSH] = W[lo:hi] * WSCALE
        wT = np.ascontiguousarray(
            Wp.reshape(VSHP, NK2, 2, P).transpose(1, 3, 2, 0).astype(f8np))
        bsl = np.full((VSHP,), -1e30, np.float32)
        bsl[:VSH] = b[lo:hi] * WSCALE
        if lo <= pad < hi:
            bsl[pad - lo] = -1e30
        b_bcast = np.ascontiguousarray(
            np.broadcast_to(bsl.astype(bF).reshape(1, VSHP), (P, VSHP)))
        a_sl = attn[c * BSH * T:(c + 1) * BSH * T]
        attnT = np.ascontiguousarray(
            a_sl.reshape(BSH * T, NS, P).transpose(1, 2, 0).astype(bF))
        s_sl = src_map[c * BSH:(c + 1) * BSH]
        srcm = np.ascontiguousarray(
            s_sl.reshape(BSH * NS, P, C).astype(bF))
        tgts = np.zeros((1, 16), np.int32)
        for g in range(len(GROUPS)):
            tgts[0, g] = 2 * NCORES * (g + 1)
        in_maps.append({
            "hT": H3,
            "wT": wT,
            "b_bcast": b_bcast,
            "w_copyT": wcT,
            "bc_neg": bcn,
            "attnT": attnT,
            "srcm": srcm,
            "hidden_cb": np.ascontiguousarray(H16[c * BSH:(c + 1) * BSH]),
            "w_copyT16": wcT16,
            "nrm_tgt": tgts,
        })
    return in_maps


def assemble(results):
    out_prob = np.concatenate(
        [np.asarray(r["out_prob"], np.float32)[:, :VSH] for r in results],
        axis=1)
    copy_prob = np.concatenate(
        [np.asarray(r["copy_prob"]).astype(np.float32) for r in results],
        axis=0)
    return np.concatenate([out_prob, copy_prob], axis=1)


FULL_CFG = dict(B=B, T=T, S=S, C=C, V=V, D=D)


def run(cfg, inputs, trace=False):
    """test.py interface: run(K.FULL_CFG, np_inputs, trace=True)."""
    nc = _get_nc()
    in_maps = make_in_maps(**inputs)
    res = run_bass_kernel_spmd(nc, in_maps, list(range(NCORES)), trace=trace)
    return assemble(res.results), res


def kernel(**inputs) -> np.ndarray:
    out, _ = run(FULL_CFG, inputs, trace=False)
    return out
